# revision 3
# baseline (speedup 1.0000x reference)
"""CenterLoss kernel for Trainium2 (8 NeuronCores, Bass/Tile).

Computation (reference):
    counts    = segment_sum(ones, y, C)                    # [C]
    feat_sum  = segment_sum(feat, y, C)                    # [C, D]
    feat_mean = feat_sum / max(counts, 1)
    ratio     = counts / (1 + counts)
    centers_grad = ratio * (centers - feat_mean)           # [C, D]
    loss      = 0.005 * sum((feat - centers[y])**2)
Note centers_grad[c] == (counts_c * centers_c - feat_sum_c) / (1 + counts_c),
identically 0 for empty classes: only occupied classes (~15k of 100k) need
compute; the rest of the output rows are zeros.

Sharding: classes range-sharded over 8 cores (core k owns [k*12500, (k+1)*12500)).
Host does index work only: sort samples by label (the all-to-all routing
permutation), route each sample's feat row to the owning core, pack each
core's samples into 128-slot tiles with no class run crossing a tile
boundary.  Device work per core:
  - indirect-DMA gather of centers[y] rows (one [128,1]-offset indirect DMA
    per 128-slot tile: hardware consumes one offset per partition-run, so
    multi-column offset APs are not usable),
  - per-tile selection matrix sel[i,j] = (y_i == y_j) on PE+DVE,
  - fsum = sel @ feat_tile on PE (each run member gets the run total),
  - counts = row-sums of sel,
  - grad rows = (n*c - fsum) * 1/(1+n), indirect-scattered per tile
    (colliding writes carry identical values -> benign),
  - masked loss partial via fused reduce + 1x1 matmul.
Host unshard: concatenate the 8 grad shards, sum the 8 loss partials.
"""

import sys

sys.path.insert(0, "/opt/trn_rl_repo")

import numpy as np

NUM_CLASSES = 100000
FEAT_DIM = 128
BATCH = 16384
N_CORES = 8
CPC = NUM_CLASSES // N_CORES  # classes per core: 12500
P = 128
# grad output shard rows: CPC + 1 trash row for pad slots, rounded up to 128
OUT_ROWS = ((CPC + 1 + P - 1) // P) * P  # 12544
LOSS_SCALE = 0.01 * 0.5

_prog_cache = {}


def _pack_core(ys_k, idx_k, G):
    """Pack one core's (class-sorted) samples into G tiles of 128 slots with
    no class run crossing a tile boundary. Returns per-slot index arrays."""
    nslots = G * P
    fidx = np.zeros(nslots, dtype=np.int64)  # sample id (for host feat routing)
    cidx = np.zeros(nslots, dtype=np.int32)  # class id to gather from centers
    sidx = np.full(nslots, CPC, dtype=np.int32)  # local scatter row (pad->trash)
    mask = np.zeros(nslots, dtype=np.float32)

    n = len(ys_k)
    if n:
        starts = np.flatnonzero(np.r_[True, ys_k[1:] != ys_k[:-1]])
        lens = np.diff(np.r_[starts, n])
        pos = 0
        for s, ln in zip(starts, lens):
            if (pos % P) + ln > P:  # run would cross tile boundary -> pad
                pos += P - (pos % P)
            assert pos + ln <= nslots, "G too small"
            fidx[pos : pos + ln] = idx_k[s : s + ln]
            cidx[pos : pos + ln] = ys_k[s]
            sidx[pos : pos + ln] = ys_k[s] % CPC
            mask[pos : pos + ln] = 1.0
            pos += ln
    # slot s = g*128 + p  ->  [p, g]
    to_pg = lambda a: a.reshape(G, P).T.copy()
    return fidx, mask, to_pg(cidx), to_pg(sidx), to_pg(mask)


def _slots_needed(ys_k):
    n = len(ys_k)
    if n == 0:
        return 0
    starts = np.flatnonzero(np.r_[True, ys_k[1:] != ys_k[:-1]])
    lens = np.diff(np.r_[starts, n])
    pos = 0
    for ln in lens:
        if (pos % P) + ln > P:
            pos += P - (pos % P)
        pos += ln
    return pos


def _build_program(G):
    import concourse.bacc as bacc
    import concourse.tile as tile
    from concourse import bass, mybir
    from concourse.masks import make_identity

    f32 = mybir.dt.float32
    i32 = mybir.dt.int32

    nc = bacc.Bacc("TRN2", target_bir_lowering=False, debug=False)

    W = G * P  # total slots
    featk_t = nc.dram_tensor("featk", [P, W], f32, kind="ExternalInput")
    cent_t = nc.dram_tensor("centers", [NUM_CLASSES, FEAT_DIM], f32, kind="ExternalInput")
    cidx_t = nc.dram_tensor("cidx", [P, G], i32, kind="ExternalInput")
    sidx_t = nc.dram_tensor("sidx", [P, G], i32, kind="ExternalInput")
    mask_t = nc.dram_tensor("mask", [P, G], f32, kind="ExternalInput")
    grad_t = nc.dram_tensor("grad", [OUT_ROWS, FEAT_DIM], f32, kind="ExternalOutput")
    loss_t = nc.dram_tensor("loss", [1, 1], f32, kind="ExternalOutput")

    ZCOL = OUT_ROWS * FEAT_DIM // P // 2  # half of grad as [P, ZCOL]

    with tile.TileContext(nc) as tc:
        with (
            tc.tile_pool(name="sb", bufs=1) as sb,
            tc.tile_pool(name="selp", bufs=3) as selp,
            tc.tile_pool(name="ps", bufs=1, space="PSUM") as ps,
            tc.tile_pool(name="tpp", bufs=2, space="PSUM") as tpp,
        ):
            # --- metadata loads (tiny) ---
            cidx = sb.tile([P, G], i32)
            sidx = sb.tile([P, G], i32)
            mask = sb.tile([P, G], f32)
            nc.sync.dma_start(cidx[:], cidx_t[:])
            nc.sync.dma_start(sidx[:], sidx_t[:])
            nc.sync.dma_start(mask[:], mask_t[:])

            # --- zero-fill the output shard (two big linear DMAs) ---
            zt = sb.tile([P, ZCOL], f32)
            nc.gpsimd.memset(zt[:], 0.0)
            gz = grad_t[:].rearrange("(p a) d -> p (a d)", p=P)
            nc.sync.dma_start(gz[:, :ZCOL], zt[:])
            nc.sync.dma_start(gz[:, ZCOL:], zt[:])

            # --- feat (host-routed, contiguous) + centers gather (per tile) ---
            fg = sb.tile([P, W], f32)
            nc.sync.dma_start(fg[:], featk_t[:])
            cg = sb.tile([P, W], f32)
            for t in range(G):
                nc.gpsimd.indirect_dma_start(
                    out=cg[:, t * P : (t + 1) * P],
                    out_offset=None,
                    in_=cent_t[:],
                    in_offset=bass.IndirectOffsetOnAxis(ap=cidx[:, t : t + 1], axis=0),
                )

            # --- loss: 0.005 * sum(mask_slot * ||f - c||^2) ---
            d = sb.tile([P, W], f32)
            d2 = sb.tile([P, W], f32)
            nc.vector.tensor_tensor(d[:], fg[:], cg[:], mybir.AluOpType.subtract)
            nc.vector.tensor_tensor(d2[:], d[:], d[:], mybir.AluOpType.mult)
            ls = sb.tile([P, G], f32)
            nc.vector.tensor_reduce(
                ls[:],
                d2[:].rearrange("p (g e) -> p g e", e=P),
                axis=mybir.AxisListType.X,
                op=mybir.AluOpType.add,
            )
            lsm = sb.tile([P, G], f32)
            nc.vector.tensor_tensor(lsm[:], ls[:], mask[:], mybir.AluOpType.mult)
            lacc = sb.tile([P, 1], f32)
            nc.vector.tensor_reduce(
                lacc[:], lsm[:], axis=mybir.AxisListType.X, op=mybir.AluOpType.add
            )
            ones = sb.tile([P, 1], f32)
            nc.gpsimd.memset(ones[:], 1.0)
            lpsum = ps.tile([1, 1], f32, tag="lpsum")
            nc.tensor.matmul(lpsum[:], lhsT=lacc[:], rhs=ones[:], start=True, stop=True)
            lsb = sb.tile([1, 1], f32)
            nc.scalar.activation(
                lsb[:], lpsum[:], mybir.ActivationFunctionType.Copy, scale=LOSS_SCALE
            )
            nc.sync.dma_start(loss_t[:], lsb[:])

            # --- per-tile selection matrix, run sums, run counts ---
            idxf = sb.tile([P, G], f32)
            nc.vector.tensor_copy(idxf[:], sidx[:])
            ident = sb.tile([P, P], f32)
            make_identity(nc, ident[:])
            nall = sb.tile([P, G], f32)

            nbank = (G + 3) // 4
            fsb = [
                ps.tile([P, 512], f32, tag=f"fs{b}", name=f"fs{b}")
                for b in range(nbank)
            ]
            for t in range(G):
                tp = tpp.tile([P, P], f32, tag="tp")
                nc.tensor.transpose(
                    tp[:], idxf[:, t : t + 1].to_broadcast([P, P]), ident[:]
                )
                sel = selp.tile([P, P], f32, tag="sel")
                nc.vector.tensor_tensor(
                    sel[:],
                    idxf[:, t : t + 1].to_broadcast([P, P]),
                    tp[:],
                    mybir.AluOpType.is_equal,
                )
                bank, slot = divmod(t, 4)
                nc.tensor.matmul(
                    fsb[bank][:, slot * P : (slot + 1) * P],
                    lhsT=sel[:],
                    rhs=fg[:, t * P : (t + 1) * P],
                    start=True,
                    stop=True,
                )
                nc.vector.tensor_reduce(
                    nall[:, t : t + 1],
                    sel[:],
                    axis=mybir.AxisListType.X,
                    op=mybir.AluOpType.add,
                )

            # inv = 1 / (1 + n)
            n1 = sb.tile([P, G], f32)
            nc.vector.tensor_scalar_add(n1[:], nall[:], 1.0)
            inv = sb.tile([P, G], f32)
            nc.vector.reciprocal(inv[:], n1[:])

            # grad = (n * c - fsum) * inv, per tile; DVE does the stt,
            # ACT does the per-partition scale; then scatter the tile's rows.
            gsb = sb.tile([P, W], f32)
            for t in range(G):
                bank, slot = divmod(t, 4)
                blk = slice(t * P, (t + 1) * P)
                nc.vector.scalar_tensor_tensor(
                    out=gsb[:, blk],
                    in0=cg[:, blk],
                    scalar=nall[:, t : t + 1],
                    in1=fsb[bank][:, slot * P : (slot + 1) * P],
                    op0=mybir.AluOpType.mult,
                    op1=mybir.AluOpType.subtract,
                )
                nc.scalar.activation(
                    gsb[:, blk],
                    gsb[:, blk],
                    mybir.ActivationFunctionType.Copy,
                    scale=inv[:, t : t + 1],
                )
            for t in range(G):
                blk = slice(t * P, (t + 1) * P)
                nc.gpsimd.indirect_dma_start(
                    out=grad_t[:],
                    out_offset=bass.IndirectOffsetOnAxis(ap=sidx[:, t : t + 1], axis=0),
                    in_=gsb[:, blk],
                    in_offset=None,
                )

    nc.compile()
    return nc


def _prepare(y, feat, centers):
    y = np.asarray(y)
    feat = np.ascontiguousarray(np.asarray(feat, dtype=np.float32))
    centers = np.ascontiguousarray(np.asarray(centers, dtype=np.float32))
    yi = y.astype(np.int64)
    order = np.argsort(yi, kind="stable").astype(np.int64)
    ys = yi[order]
    bounds = np.searchsorted(ys, np.arange(N_CORES + 1) * CPC)

    G = 1
    for k in range(N_CORES):
        need = _slots_needed(ys[bounds[k] : bounds[k + 1]])
        G = max(G, (need + P - 1) // P)

    in_maps = []
    for k in range(N_CORES):
        ys_k = ys[bounds[k] : bounds[k + 1]]
        idx_k = order[bounds[k] : bounds[k + 1]]
        fidx, mask_flat, cidx, sidx, mask = _pack_core(ys_k, idx_k, G)
        # featk[p, g*128:(g+1)*128] = feat[fidx[slot g*128+p]] (0 for pads)
        featk = feat[fidx] * mask_flat[:, None]  # [W, 128]
        featk = (
            featk.reshape(G, P, FEAT_DIM).transpose(1, 0, 2).reshape(P, G * FEAT_DIM)
        )
        in_maps.append(
            {
                "featk": np.ascontiguousarray(featk),
                "centers": centers,
                "cidx": cidx,
                "sidx": sidx,
                "mask": mask,
            }
        )
    return G, in_maps


def kernel_run(y, feat, centers, trace=False):
    """Returns ((loss, centers_grad), BassKernelResults)."""
    from concourse.bass_utils import run_bass_kernel_spmd

    G, in_maps = _prepare(y, feat, centers)
    if G not in _prog_cache:
        _prog_cache[G] = _build_program(G)
    nc = _prog_cache[G]

    res = run_bass_kernel_spmd(
        nc, in_maps, core_ids=list(range(N_CORES)), trace=trace
    )
    grad = np.concatenate([res.results[k]["grad"][:CPC] for k in range(N_CORES)], axis=0)
    loss = np.float32(sum(float(res.results[k]["loss"][0, 0]) for k in range(N_CORES)))
    return (loss, grad), res


def kernel(y, feat, centers):
    out, _ = kernel_run(y, feat, centers, trace=False)
    return out


# revision 7
# speedup vs baseline: 1.2026x; 1.2026x over previous
"""CenterLoss kernel for Trainium2 (8 NeuronCores, Bass/Tile).

Computation (reference):
    counts    = segment_sum(ones, y, C)                    # [C]
    feat_sum  = segment_sum(feat, y, C)                    # [C, D]
    feat_mean = feat_sum / max(counts, 1)
    ratio     = counts / (1 + counts)
    centers_grad = ratio * (centers - feat_mean)           # [C, D]
    loss      = 0.005 * sum((feat - centers[y])**2)
centers_grad[c] == (counts_c * centers_c - feat_sum_c) / (1 + counts_c),
identically 0 for empty classes: only occupied classes (~15k of 100k) need
compute; the rest of the output rows stay zero.

Sharding: classes range-sharded over 8 cores (core k owns [k*12500,(k+1)*12500)).
Host does index work only: sort samples by label (the all-to-all routing
permutation), route each sample's feat row to the owning core, and pack each
core's samples into 128-slot tiles with no class run crossing a tile boundary.
Device work per core (slot s = g*128 + p lives at SBUF [p, g*128:(g+1)*128]):
  - dma_gather of centers rows from the core's class shard (one bulk SWDGE
    instruction, int16 shard-local indices; pad slots index an appended
    zero row so no masking is needed anywhere),
  - per-tile selection matrix sel[i,j] = (y_i == y_j) on PE+DVE,
  - fsum = sel @ feat_tile on PE (each run member gets the run total),
  - counts = row-sums of sel,
  - grad rows = (n*c - fsum) * 1/(1+n),
  - one bulk dma_scatter_add into the zero-filled output shard: run-last
    slots add the full grad row to their class row, all other slots add
    into a trash row that is dropped on unshard,
  - loss partial: fused (f-c)^2 reduce + 1x1 matmul (pads contribute 0).
Host unshard: concatenate the 8 grad shards, sum the 8 loss partials.
"""

import sys

sys.path.insert(0, "/opt/trn_rl_repo")

import numpy as np

NUM_CLASSES = 100000
FEAT_DIM = 128
BATCH = 16384
N_CORES = 8
CPC = NUM_CLASSES // N_CORES  # classes per core: 12500
P = 128
SHARD_ROWS = CPC + 1  # + zero row for pad gathers
OUT_ROWS = ((CPC + 1 + P - 1) // P) * P  # 12544; row OUT_ROWS-1 is the trash row
TRASH = OUT_ROWS - 1
LOSS_SCALE = 0.01 * 0.5

_prog_cache = {}


def _pack_core(ys_k, idx_k, G):
    """Pack one core's (class-sorted) samples into G tiles of 128 slots with
    no class run crossing a tile boundary."""
    nslots = G * P
    fidx = np.zeros(nslots, dtype=np.int64)  # sample id (host feat routing)
    valid = np.zeros(nslots, dtype=np.float32)
    cidx = np.full(nslots, CPC, dtype=np.int16)  # gather idx (pad -> zero row)
    sidx = np.full(nslots, TRASH, dtype=np.int16)  # scatter idx (default trash)

    n = len(ys_k)
    if n:
        starts = np.flatnonzero(np.r_[True, ys_k[1:] != ys_k[:-1]])
        lens = np.diff(np.r_[starts, n])
        pos = 0
        for s, ln in zip(starts, lens):
            if (pos % P) + ln > P:  # run would cross tile boundary -> pad
                pos += P - (pos % P)
            assert pos + ln <= nslots, "G too small"
            fidx[pos : pos + ln] = idx_k[s : s + ln]
            valid[pos : pos + ln] = 1.0
            cidx[pos : pos + ln] = ys_k[s] % CPC
            sidx[pos + ln - 1] = ys_k[s] % CPC  # run-last slot scatters
            pos += ln
    return fidx, valid, cidx, sidx


def _slots_needed(ys_k):
    n = len(ys_k)
    if n == 0:
        return 0
    starts = np.flatnonzero(np.r_[True, ys_k[1:] != ys_k[:-1]])
    lens = np.diff(np.r_[starts, n])
    pos = 0
    for ln in lens:
        if (pos % P) + ln > P:
            pos += P - (pos % P)
        pos += ln
    return pos


def _wrap_idxs(a):
    """[nslots] int16 -> [128, nslots/16] idx tile (16-wrap, replicated 8x)."""
    n = len(a)
    w = a.reshape(n // 16, 16).T  # idx i at [i%16, i//16]
    return np.ascontiguousarray(np.tile(w, (8, 1)))


def _build_program(G):
    import concourse.bacc as bacc
    import concourse.tile as tile
    from concourse import bass, mybir
    from concourse.masks import make_identity

    f32 = mybir.dt.float32
    i16 = mybir.dt.int16

    nc = bacc.Bacc("TRN2", target_bir_lowering=False, debug=False)

    W = G * P  # total slots
    IW = W // 16  # idx tile width
    featk_t = nc.dram_tensor("featk", [P, W], f32, kind="ExternalInput")
    centk_t = nc.dram_tensor("centk", [SHARD_ROWS, FEAT_DIM], f32, kind="ExternalInput")
    cidx_t = nc.dram_tensor("cidx", [P, IW], i16, kind="ExternalInput")
    sidx_t = nc.dram_tensor("sidx", [P, IW], i16, kind="ExternalInput")
    grad_t = nc.dram_tensor("grad", [OUT_ROWS, FEAT_DIM], f32, kind="ExternalOutput")
    loss_t = nc.dram_tensor("loss", [1, 1], f32, kind="ExternalOutput")

    ZCOL = OUT_ROWS * FEAT_DIM // P // 2  # half of grad as [P, ZCOL]

    with tile.TileContext(nc) as tc:
        with (
            tc.tile_pool(name="sb", bufs=1) as sb,
            tc.tile_pool(name="selp", bufs=3) as selp,
            tc.tile_pool(name="ps", bufs=1, space="PSUM") as ps,
            tc.tile_pool(name="tpp", bufs=2, space="PSUM") as tpp,
        ):
            # --- metadata loads (tiny) ---
            cidx = sb.tile([P, IW], i16)
            sidx = sb.tile([P, IW], i16)
            nc.sync.dma_start(cidx[:], cidx_t[:])
            nc.sync.dma_start(sidx[:], sidx_t[:])

            # --- zero-fill the output shard (two big linear DMAs) ---
            zt = sb.tile([P, ZCOL], f32)
            nc.gpsimd.memset(zt[:], 0.0)
            gz = grad_t[:].rearrange("(p a) d -> p (a d)", p=P)
            nc.sync.dma_start(gz[:, :ZCOL], zt[:])
            nc.sync.dma_start(gz[:, ZCOL:], zt[:])

            # --- feat (host-routed, contiguous) + centers bulk gather ---
            fg = sb.tile([P, W], f32)
            nc.sync.dma_start(fg[:], featk_t[:])
            cg = sb.tile([P, W], f32)
            nc.gpsimd.dma_gather(
                out_ap=cg[:].rearrange("p (g e) -> p g e", e=P),
                in_ap=centk_t[:],
                idxs_ap=cidx[:],
                num_idxs=W,
                num_idxs_reg=W,
                elem_size=FEAT_DIM,
                single_packet=False,
            )

            # --- loss: 0.005 * sum((f - c)^2); pads have f=0, c=0 ---
            d = sb.tile([P, W], f32)
            d2 = sb.tile([P, W], f32)
            lacc = sb.tile([P, 1], f32)
            nc.vector.tensor_tensor(d[:], fg[:], cg[:], mybir.AluOpType.subtract)
            nc.vector.tensor_tensor(d2[:], d[:], d[:], mybir.AluOpType.mult)
            nc.vector.tensor_reduce(
                lacc[:], d2[:], axis=mybir.AxisListType.X, op=mybir.AluOpType.add
            )
            ones = sb.tile([P, 1], f32)
            nc.gpsimd.memset(ones[:], 1.0)
            lpsum = ps.tile([1, 1], f32, tag="lpsum")
            nc.tensor.matmul(lpsum[:], lhsT=lacc[:], rhs=ones[:], start=True, stop=True)
            lsb = sb.tile([1, 1], f32)
            nc.scalar.activation(
                lsb[:], lpsum[:], mybir.ActivationFunctionType.Copy, scale=LOSS_SCALE
            )
            nc.sync.dma_start(loss_t[:], lsb[:])

            # --- per-tile selection matrix, run sums, run counts ---
            # class-id-per-slot for the selection compare comes from the
            # gathered idx tile: unwrap cidx int16 -> f32 per slot
            idxf = sb.tile([P, G], f32)
            # cidx[p, :] holds idxs wrapped by 16; slot (p,g)'s class id is at
            # cidx[(g*128+p) % 16 ... ] -- awkward. Instead load class ids per
            # slot directly from featk? Simpler: separate tiny input.
            # (filled below via clsf_t)
            clsf_t = nc.dram_tensor("clsf", [P, G], f32, kind="ExternalInput")
            nc.sync.dma_start(idxf[:], clsf_t[:])
            ident = sb.tile([P, P], f32)
            make_identity(nc, ident[:])
            nall = sb.tile([P, G], f32)

            nbank = (G + 3) // 4
            fsb = [
                ps.tile([P, 512], f32, tag=f"fs{b}", name=f"fs{b}")
                for b in range(nbank)
            ]
            for t in range(G):
                tp = tpp.tile([P, P], f32, tag="tp")
                nc.tensor.transpose(
                    tp[:], idxf[:, t : t + 1].to_broadcast([P, P]), ident[:]
                )
                sel = selp.tile([P, P], f32, tag="sel")
                nc.vector.tensor_tensor(
                    sel[:],
                    idxf[:, t : t + 1].to_broadcast([P, P]),
                    tp[:],
                    mybir.AluOpType.is_equal,
                )
                bank, slot = divmod(t, 4)
                nc.tensor.matmul(
                    fsb[bank][:, slot * P : (slot + 1) * P],
                    lhsT=sel[:],
                    rhs=fg[:, t * P : (t + 1) * P],
                    start=True,
                    stop=True,
                )
                nc.vector.tensor_reduce(
                    nall[:, t : t + 1],
                    sel[:],
                    axis=mybir.AxisListType.X,
                    op=mybir.AluOpType.add,
                )

            # inv = 1 / (1 + n)
            n1 = sb.tile([P, G], f32)
            nc.vector.tensor_scalar_add(n1[:], nall[:], 1.0)
            inv = sb.tile([P, G], f32)
            nc.vector.reciprocal(inv[:], n1[:])

            # grad = (n * c - fsum) * inv per tile: DVE stt + ACT scale
            gsb = sb.tile([P, W], f32)
            for t in range(G):
                bank, slot = divmod(t, 4)
                blk = slice(t * P, (t + 1) * P)
                nc.vector.scalar_tensor_tensor(
                    out=gsb[:, blk],
                    in0=cg[:, blk],
                    scalar=nall[:, t : t + 1],
                    in1=fsb[bank][:, slot * P : (slot + 1) * P],
                    op0=mybir.AluOpType.mult,
                    op1=mybir.AluOpType.subtract,
                )
                nc.scalar.activation(
                    gsb[:, blk],
                    gsb[:, blk],
                    mybir.ActivationFunctionType.Copy,
                    scale=inv[:, t : t + 1],
                )

            # --- bulk scatter-add into the zeroed shard ---
            nc.gpsimd.dma_scatter_add(
                out_ap=grad_t[:],
                in_ap=gsb[:].rearrange("p (g e) -> p g e", e=P),
                idxs_ap=sidx[:],
                num_idxs=W,
                num_idxs_reg=W,
                elem_size=FEAT_DIM,
                single_packet=False,
            )

    nc.compile()
    return nc


def _prepare(y, feat, centers):
    y = np.asarray(y)
    feat = np.ascontiguousarray(np.asarray(feat, dtype=np.float32))
    centers = np.ascontiguousarray(np.asarray(centers, dtype=np.float32))
    yi = y.astype(np.int64)
    order = np.argsort(yi, kind="stable").astype(np.int64)
    ys = yi[order]
    bounds = np.searchsorted(ys, np.arange(N_CORES + 1) * CPC)

    G = 1
    for k in range(N_CORES):
        need = _slots_needed(ys[bounds[k] : bounds[k + 1]])
        G = max(G, (need + P - 1) // P)

    zrow = np.zeros((1, FEAT_DIM), np.float32)
    in_maps = []
    for k in range(N_CORES):
        ys_k = ys[bounds[k] : bounds[k + 1]]
        idx_k = order[bounds[k] : bounds[k + 1]]
        fidx, valid, cidx, sidx = _pack_core(ys_k, idx_k, G)
        # featk[p, g*128:(g+1)*128] = feat[fidx[slot g*128+p]] (0 for pads)
        featk = feat[fidx] * valid[:, None]  # [W, 128]
        featk = (
            featk.reshape(G, P, FEAT_DIM).transpose(1, 0, 2).reshape(P, G * FEAT_DIM)
        )
        centk = np.ascontiguousarray(
            np.vstack([centers[k * CPC : (k + 1) * CPC], zrow])
        )
        in_maps.append(
            {
                "featk": np.ascontiguousarray(featk),
                "centk": centk,
                "cidx": _wrap_idxs(cidx),
                "sidx": _wrap_idxs(sidx),
                "clsf": np.ascontiguousarray(
                    cidx.astype(np.float32).reshape(G, P).T
                ),
            }
        )
    return G, in_maps


def kernel_run(y, feat, centers, trace=False):
    """Returns ((loss, centers_grad), BassKernelResults)."""
    from concourse.bass_utils import run_bass_kernel_spmd

    G, in_maps = _prepare(y, feat, centers)
    if G not in _prog_cache:
        _prog_cache[G] = _build_program(G)
    nc = _prog_cache[G]

    res = run_bass_kernel_spmd(
        nc, in_maps, core_ids=list(range(N_CORES)), trace=trace
    )
    grad = np.concatenate([res.results[k]["grad"][:CPC] for k in range(N_CORES)], axis=0)
    loss = np.float32(sum(float(res.results[k]["loss"][0, 0]) for k in range(N_CORES)))
    return (loss, grad), res


def kernel(y, feat, centers):
    out, _ = kernel_run(y, feat, centers, trace=False)
    return out


# revision 10
# speedup vs baseline: 1.5968x; 1.3277x over previous
"""CenterLoss kernel for Trainium2 (8 NeuronCores, Bass/Tile).

Computation (reference):
    counts    = segment_sum(ones, y, C)                    # [C]
    feat_sum  = segment_sum(feat, y, C)                    # [C, D]
    feat_mean = feat_sum / max(counts, 1)
    ratio     = counts / (1 + counts)
    centers_grad = ratio * (centers - feat_mean)           # [C, D]
    loss      = 0.005 * sum((feat - centers[y])**2)
centers_grad[c] == (counts_c * centers_c - feat_sum_c) / (1 + counts_c),
identically 0 for empty classes: only occupied classes (~15k of 100k) need
compute; the rest of the output rows stay zero.

Sharding: classes range-sharded over 8 cores (core k owns [k*12500,(k+1)*12500)).
Host does index work only: sort samples by label (the all-to-all routing
permutation), route each sample's feat row to the owning core, and pack each
core's samples into 128-slot tiles with no class run crossing a tile boundary.
Device work per core (slot s = g*128 + p lives at SBUF [p, g*128:(g+1)*128]):
  - bulk dma_gather of centers rows from the core's class shard (int16
    shard-local indices; pad slots index an appended zero row so no masking
    is needed anywhere), chunked 4x so Q7 descriptor generation pipelines
    with compute; a 16-index warmup gather/scatter pair runs first so the
    Q7 ucode library loads off the critical path,
  - per-tile selection matrix sel[i,j] = (y_i == y_j) on PE+DVE,
  - fsum = sel @ feat_tile on PE (each run member gets the run total),
  - counts = row-sums of sel,
  - grad rows = (n*c - fsum) * 1/(1+n),
  - per-chunk dma_scatter_add into the zero-filled output shard: run-last
    slots add the full grad row to their class row, all other slots add
    into a trash row dropped on unshard,
  - loss partial per chunk: DVE sub + ACT Square-with-accumulate, then one
    small matmul against ones.
Host unshard: concatenate the 8 grad shards, sum the per-chunk loss partials.
"""

import sys

sys.path.insert(0, "/opt/trn_rl_repo")

import numpy as np

NUM_CLASSES = 100000
FEAT_DIM = 128
BATCH = 16384
N_CORES = 8
CPC = NUM_CLASSES // N_CORES  # classes per core: 12500
P = 128
SHARD_ROWS = CPC + 1  # + zero row for pad gathers
OUT_ROWS = ((CPC + 1 + P - 1) // P) * P  # 12544; row OUT_ROWS-1 is the trash row
TRASH = OUT_ROWS - 1
LOSS_SCALE = 0.01 * 0.5

_prog_cache = {}


def _chunks(G):
    n = min(4, G)
    base, rem = divmod(G, n)
    sizes = [base + (1 if i < rem else 0) for i in range(n)]
    offs = np.cumsum([0] + sizes).tolist()
    return n, sizes, offs


def _pack_core(ys_k, idx_k, G):
    """Pack one core's (class-sorted) samples into G tiles of 128 slots with
    no class run crossing a tile boundary."""
    nslots = G * P
    fidx = np.zeros(nslots, dtype=np.int64)  # sample id (host feat routing)
    valid = np.zeros(nslots, dtype=np.float32)
    cidx = np.full(nslots, CPC, dtype=np.int16)  # gather idx (pad -> zero row)
    sidx = np.full(nslots, TRASH, dtype=np.int16)  # scatter idx (default trash)

    n = len(ys_k)
    if n:
        starts = np.flatnonzero(np.r_[True, ys_k[1:] != ys_k[:-1]])
        lens = np.diff(np.r_[starts, n])
        pos = 0
        for s, ln in zip(starts, lens):
            if (pos % P) + ln > P:  # run would cross tile boundary -> pad
                pos += P - (pos % P)
            assert pos + ln <= nslots, "G too small"
            fidx[pos : pos + ln] = idx_k[s : s + ln]
            valid[pos : pos + ln] = 1.0
            cidx[pos : pos + ln] = ys_k[s] % CPC
            sidx[pos + ln - 1] = ys_k[s] % CPC  # run-last slot scatters
            pos += ln
    return fidx, valid, cidx, sidx


def _slots_needed(ys_k):
    n = len(ys_k)
    if n == 0:
        return 0
    starts = np.flatnonzero(np.r_[True, ys_k[1:] != ys_k[:-1]])
    lens = np.diff(np.r_[starts, n])
    pos = 0
    for ln in lens:
        if (pos % P) + ln > P:
            pos += P - (pos % P)
        pos += ln
    return pos


def _wrap_idxs(a):
    """[n] int16 -> [128, n/16] idx tile (16-wrap, replicated 8x)."""
    n = len(a)
    w = a.reshape(n // 16, 16).T  # idx i at [i%16, i//16]
    return np.ascontiguousarray(np.tile(w, (8, 1)))


def _build_program(G):
    import concourse.bacc as bacc
    import concourse.tile as tile
    from concourse import bass, mybir

    f32 = mybir.dt.float32
    i16 = mybir.dt.int16

    NCH, sizes, offs = _chunks(G)
    W = G * P

    nc = bacc.Bacc("TRN2", target_bir_lowering=False, debug=False)

    featk_t = nc.dram_tensor("featk", [P, W], f32, kind="ExternalInput")
    centk_t = nc.dram_tensor("centk", [SHARD_ROWS, FEAT_DIM], f32, kind="ExternalInput")
    ident_t = nc.dram_tensor("ident", [P, P], f32, kind="ExternalInput")
    clsf_t = nc.dram_tensor("clsf", [P, G], f32, kind="ExternalInput")
    cidx_ts, sidx_ts = [], []
    for i in range(NCH):
        wi = sizes[i] * P
        cidx_ts.append(nc.dram_tensor(f"cidx{i}", [P, wi // 16], i16, kind="ExternalInput"))
        sidx_ts.append(nc.dram_tensor(f"sidx{i}", [P, wi // 16], i16, kind="ExternalInput"))
    grad_t = nc.dram_tensor("grad", [OUT_ROWS, FEAT_DIM], f32, kind="ExternalOutput")
    loss_t = nc.dram_tensor("loss", [NCH, 1], f32, kind="ExternalOutput")

    NZ = 4  # zero-fill DMA count
    ZCOL = OUT_ROWS * FEAT_DIM // P // NZ

    with tile.TileContext(nc) as tc:
        with (
            tc.tile_pool(name="sb", bufs=1) as sb,
            tc.tile_pool(name="selp", bufs=3) as selp,
            tc.tile_pool(name="ps", bufs=1, space="PSUM") as ps,
            tc.tile_pool(name="tpp", bufs=2, space="PSUM") as tpp,
            tc.tile_pool(name="dramp", bufs=1, space="DRAM") as dramp,
        ):
            # --- metadata loads (tiny) ---
            cidxs, sidxs = [], []
            for i in range(NCH):
                wi = sizes[i] * P
                ci = sb.tile([P, wi // 16], i16, tag=f"ci{i}", name=f"ci{i}")
                si = sb.tile([P, wi // 16], i16, tag=f"si{i}", name=f"si{i}")
                nc.sync.dma_start(ci[:], cidx_ts[i][:])
                nc.sync.dma_start(si[:], sidx_ts[i][:])
                cidxs.append(ci)
                sidxs.append(si)
            idxf = sb.tile([P, G], f32)
            nc.sync.dma_start(idxf[:], clsf_t[:])
            ident = sb.tile([P, P], f32)
            nc.sync.dma_start(ident[:], ident_t[:])
            fg = sb.tile([P, W], f32)
            nc.sync.dma_start(fg[:], featk_t[:])

            # --- ucode warmup: tiny gather + tiny scatter-add ---
            widx = sb.tile([P, 1], i16)
            nc.vector.memset(widx[:], CPC)  # zero row
            wg = sb.tile([P, P], f32)
            nc.gpsimd.dma_gather(
                out_ap=wg[:].rearrange("p (g e) -> p g e", e=P),
                in_ap=centk_t[:],
                idxs_ap=widx[:],
                num_idxs=16,
                num_idxs_reg=16,
                elem_size=FEAT_DIM,
                single_packet=False,
            )
            widx0 = sb.tile([P, 1], i16)
            nc.vector.memset(widx0[:], 0)
            ws = sb.tile([16, 8 * P], f32)
            nc.vector.memset(ws[:], 0.0)
            dum = dramp.tile([P, P], f32)
            nc.gpsimd.dma_scatter_add(
                out_ap=dum[:],
                in_ap=ws[:].rearrange("p (g e) -> p g e", e=P),
                idxs_ap=widx0[:],
                num_idxs=16,
                num_idxs_reg=16,
                elem_size=FEAT_DIM,
                single_packet=False,
            )

            # --- gathers (chunked) ---
            cg = sb.tile([P, W], f32)
            for i in range(NCH):
                lo, wi = offs[i] * P, sizes[i] * P
                nc.gpsimd.dma_gather(
                    out_ap=cg[:, lo : lo + wi].rearrange("p (g e) -> p g e", e=P),
                    in_ap=centk_t[:],
                    idxs_ap=cidxs[i][:],
                    num_idxs=wi,
                    num_idxs_reg=wi,
                    elem_size=FEAT_DIM,
                    single_packet=False,
                )

            # --- zero-fill the output shard ---
            zt = sb.tile([P, ZCOL], f32)
            nc.vector.memset(zt[:], 0.0)
            gz = grad_t[:].rearrange("(p a) d -> p (a d)", p=P)
            for z in range(NZ):
                nc.sync.dma_start(gz[:, z * ZCOL : (z + 1) * ZCOL], zt[:])

            ones = sb.tile([P, 1], f32)
            nc.vector.memset(ones[:], 1.0)

            # --- per-chunk compute + scatter ---
            d = sb.tile([P, W], f32)
            d2 = sb.tile([P, W], f32)
            lacc = sb.tile([P, NCH], f32)
            nall = sb.tile([P, G], f32)
            n1 = sb.tile([P, G], f32)
            inv = sb.tile([P, G], f32)
            gsb = sb.tile([P, W], f32)
            nbank = (G + 3) // 4
            fsb = [
                ps.tile([P, 512], f32, tag=f"fs{b}", name=f"fs{b}")
                for b in range(nbank)
            ]

            for i in range(NCH):
                t0, t1 = offs[i], offs[i + 1]
                lo, hi = t0 * P, t1 * P
                # loss partial
                nc.vector.tensor_tensor(
                    d[:, lo:hi], fg[:, lo:hi], cg[:, lo:hi], mybir.AluOpType.subtract
                )
                nc.scalar.activation(
                    d2[:, lo:hi],
                    d[:, lo:hi],
                    mybir.ActivationFunctionType.Square,
                    accum_out=lacc[:, i : i + 1],
                )
                # selection matrices, run sums, counts
                for t in range(t0, t1):
                    tp = tpp.tile([P, P], f32, tag="tp")
                    nc.tensor.transpose(
                        tp[:], idxf[:, t : t + 1].to_broadcast([P, P]), ident[:]
                    )
                    sel = selp.tile([P, P], f32, tag="sel")
                    nc.vector.tensor_tensor(
                        sel[:],
                        idxf[:, t : t + 1].to_broadcast([P, P]),
                        tp[:],
                        mybir.AluOpType.is_equal,
                    )
                    bank, slot = divmod(t, 4)
                    nc.tensor.matmul(
                        fsb[bank][:, slot * P : (slot + 1) * P],
                        lhsT=sel[:],
                        rhs=fg[:, t * P : (t + 1) * P],
                        start=True,
                        stop=True,
                    )
                    nc.vector.tensor_reduce(
                        nall[:, t : t + 1],
                        sel[:],
                        axis=mybir.AxisListType.X,
                        op=mybir.AluOpType.add,
                    )
                # inv = 1/(1+n) for this chunk
                nc.vector.tensor_scalar_add(n1[:, t0:t1], nall[:, t0:t1], 1.0)
                nc.vector.reciprocal(inv[:, t0:t1], n1[:, t0:t1])
                # grad rows
                for t in range(t0, t1):
                    bank, slot = divmod(t, 4)
                    blk = slice(t * P, (t + 1) * P)
                    nc.vector.scalar_tensor_tensor(
                        out=gsb[:, blk],
                        in0=cg[:, blk],
                        scalar=nall[:, t : t + 1],
                        in1=fsb[bank][:, slot * P : (slot + 1) * P],
                        op0=mybir.AluOpType.mult,
                        op1=mybir.AluOpType.subtract,
                    )
                    nc.scalar.activation(
                        gsb[:, blk],
                        gsb[:, blk],
                        mybir.ActivationFunctionType.Copy,
                        scale=inv[:, t : t + 1],
                    )
                # scatter this chunk
                nc.gpsimd.dma_scatter_add(
                    out_ap=grad_t[:],
                    in_ap=gsb[:, lo:hi].rearrange("p (g e) -> p g e", e=P),
                    idxs_ap=sidxs[i][:],
                    num_idxs=hi - lo,
                    num_idxs_reg=hi - lo,
                    elem_size=FEAT_DIM,
                    single_packet=False,
                )

            # --- loss total: [NCH,1] = lacc^T @ ones, scaled ---
            lpsum = ps.tile([NCH, 1], f32, tag="lpsum")
            nc.tensor.matmul(lpsum[:], lhsT=lacc[:], rhs=ones[:], start=True, stop=True)
            lsb = sb.tile([NCH, 1], f32)
            nc.scalar.activation(
                lsb[:], lpsum[:], mybir.ActivationFunctionType.Copy, scale=LOSS_SCALE
            )
            nc.sync.dma_start(loss_t[:], lsb[:])

    nc.compile()
    return nc


def _prepare(y, feat, centers):
    y = np.asarray(y)
    feat = np.ascontiguousarray(np.asarray(feat, dtype=np.float32))
    centers = np.ascontiguousarray(np.asarray(centers, dtype=np.float32))
    yi = y.astype(np.int64)
    order = np.argsort(yi, kind="stable").astype(np.int64)
    ys = yi[order]
    bounds = np.searchsorted(ys, np.arange(N_CORES + 1) * CPC)

    G = 1
    for k in range(N_CORES):
        need = _slots_needed(ys[bounds[k] : bounds[k + 1]])
        G = max(G, (need + P - 1) // P)

    NCH, sizes, offs = _chunks(G)
    zrow = np.zeros((1, FEAT_DIM), np.float32)
    ident = np.eye(P, dtype=np.float32)
    in_maps = []
    for k in range(N_CORES):
        ys_k = ys[bounds[k] : bounds[k + 1]]
        idx_k = order[bounds[k] : bounds[k + 1]]
        fidx, valid, cidx, sidx = _pack_core(ys_k, idx_k, G)
        featk = feat[fidx] * valid[:, None]  # [W, 128]
        featk = (
            featk.reshape(G, P, FEAT_DIM).transpose(1, 0, 2).reshape(P, G * FEAT_DIM)
        )
        centk = np.ascontiguousarray(
            np.vstack([centers[k * CPC : (k + 1) * CPC], zrow])
        )
        im = {
            "featk": np.ascontiguousarray(featk),
            "centk": centk,
            "ident": ident,
            "clsf": np.ascontiguousarray(cidx.astype(np.float32).reshape(G, P).T),
        }
        for i in range(NCH):
            lo, hi = offs[i] * P, offs[i + 1] * P
            im[f"cidx{i}"] = _wrap_idxs(cidx[lo:hi])
            im[f"sidx{i}"] = _wrap_idxs(sidx[lo:hi])
        in_maps.append(im)
    return G, in_maps


def kernel_run(y, feat, centers, trace=False):
    """Returns ((loss, centers_grad), BassKernelResults)."""
    from concourse.bass_utils import run_bass_kernel_spmd

    G, in_maps = _prepare(y, feat, centers)
    if G not in _prog_cache:
        _prog_cache[G] = _build_program(G)
    nc = _prog_cache[G]

    res = run_bass_kernel_spmd(
        nc, in_maps, core_ids=list(range(N_CORES)), trace=trace
    )
    grad = np.concatenate([res.results[k]["grad"][:CPC] for k in range(N_CORES)], axis=0)
    loss = np.float32(
        sum(float(res.results[k]["loss"].sum()) for k in range(N_CORES))
    )
    return (loss, grad), res


def kernel(y, feat, centers):
    out, _ = kernel_run(y, feat, centers, trace=False)
    return out
